# revision 1
# baseline (speedup 1.0000x reference)
"""Trainium2 Bass kernel for nn_BaseGCNModel_addSE (gnn_message_passing).

SPMD over 8 NeuronCores, data laid out so the SE gate commutes with the
sparse aggregation:

    agg = A @ (x * (1+gate)) = (A @ x) * (1+gate)

since the gate is constant along the contracted node axis. So the kernel
gathers messages directly from the host-marshalled node-major table
xt [N, B*F] (fp16, 1KB rows), segment-sums them on the PE via streamed
fp16 one-hot blocks (edge weights folded in), and applies the gate as a
per-partition scalar fused into the PSUM-evacuation of the transposed
aggregates. Every core owns 16 of the 128 dst-node tiles; BN1 is
node-local; one tiny AllReduce(max) combines pooled partials; the FC
head runs replicated.

Host side only marshals: transposes/casts x, sorts edges by dst, pads
per 128-node tile to a uniform chunk count, builds int16 gather-index
tables and fp16 one-hot blocks.
"""

import os
import sys

for _p in ("/opt/trn_rl_repo", "/root/.axon_site/_ro/trn_rl_repo"):
    if _p not in sys.path:
        sys.path.insert(0, _p)

import numpy as np

import concourse.bass as bass
import concourse.bacc as bacc
import concourse.mybir as mybir
import concourse.tile as tile
from concourse.bass_utils import run_bass_kernel_spmd
from concourse.masks import make_identity

f16 = np.float16
F32 = mybir.dt.float32
F16 = mybir.dt.float16
I16 = mybir.dt.int16
AF = mybir.ActivationFunctionType
ALU = mybir.AluOpType
AX = mybir.AxisListType

B, N, F, E, H = 8, 16384, 64, 262144, 128
SE_D = 32
FC1, FC2, OUT = 256, 128, 4
BN_EPS = 1e-3
NCORES = 8
NTILE = 128            # global 128-node dst tiles
TPC = NTILE // NCORES  # dst tiles per core (16)
BF = B * F             # 512, xt row width
MAX_GATHER = 1024      # SWDGE ring limit: >1024 descriptors per gather crashes


def build_kernel(cpts, skip_collective: bool = False, phases: str = "GB"):
    """Build the SPMD program. cpts[i] = chunks (of 128 edges) for tile slot i
    (per-core tiles are sorted by descending edge count, so slot i's static
    size is the max of the i-th order statistic across cores)."""
    if isinstance(cpts, int):
        cpts = (cpts,) * TPC
    slots_i = [c * 128 for c in cpts]
    total_slots = sum(slots_i)
    offs_i = np.concatenate([[0], np.cumsum(slots_i)]).astype(int)
    nc = bacc.Bacc("TRN2", target_bir_lowering=False, debug=False,
                   num_devices=NCORES)

    # inputs (identical content on every core unless noted "per-core")
    xt = nc.dram_tensor("xt", [N, BF], F16, kind="ExternalInput")
    xs = nc.dram_tensor("xs", [N // NCORES, BF], F16, kind="ExternalInput")  # per-core x slice
    gidx = nc.dram_tensor("gidx", [128, total_slots // 16], I16, kind="ExternalInput")  # per-core
    smat = nc.dram_tensor("smat", [128, total_slots], F16, kind="ExternalInput")         # per-core
    bn1p = nc.dram_tensor("bn1p", [TPC, 128, 2], F32, kind="ExternalInput")             # per-core
    w1 = nc.dram_tensor("w1", [F, SE_D], F32, kind="ExternalInput")
    b1 = nc.dram_tensor("b1", [SE_D, 1], F32, kind="ExternalInput")
    w2 = nc.dram_tensor("w2", [SE_D, SE_D], F32, kind="ExternalInput")
    b2 = nc.dram_tensor("b2", [SE_D, 1], F32, kind="ExternalInput")
    wop = nc.dram_tensor("wop", [SE_D, F], F32, kind="ExternalInput")
    bop = nc.dram_tensor("bop", [F, 1], F32, kind="ExternalInput")
    wge = nc.dram_tensor("wge", [F + 1, H], F32, kind="ExternalInput")  # [Wg; bg]
    wf1 = nc.dram_tensor("wf1", [H, FC1], F32, kind="ExternalInput")
    bf1r = nc.dram_tensor("bf1r", [1, FC1], F32, kind="ExternalInput")
    wf2 = nc.dram_tensor("wf2", [2, H, FC2], F32, kind="ExternalInput")
    bf2r = nc.dram_tensor("bf2r", [1, FC2], F32, kind="ExternalInput")
    wo = nc.dram_tensor("wo", [FC2, OUT], F32, kind="ExternalInput")
    bor = nc.dram_tensor("bor", [1, OUT], F32, kind="ExternalInput")
    g2c = nc.dram_tensor("g2c", [H, 2], F32, kind="ExternalInput")
    be2c = nc.dram_tensor("be2c", [H, 2], F32, kind="ExternalInput")
    g3c = nc.dram_tensor("g3c", [FC2, 1], F32, kind="ExternalInput")
    be3c = nc.dram_tensor("be3c", [FC2, 1], F32, kind="ExternalInput")

    out_t = nc.dram_tensor("out", [B, OUT], F32, kind="ExternalOutput")

    with tile.TileContext(nc) as tc:
        with (
            tc.tile_pool(name="const", bufs=1) as cpool,
            tc.tile_pool(name="sbuf", bufs=2) as pool,
            tc.tile_pool(name="psum", bufs=2, space="PSUM") as psum,
            tc.tile_pool(name="dram", bufs=1, space="DRAM") as dpool,
        ):
            # ---- constants / weights ----
            ident = cpool.tile([128, 128], F32)
            make_identity(nc, ident[:])
            ones_r = cpool.tile([1, 128], F32)
            nc.vector.memset(ones_r[:], 1.0)

            def load_const(shape, src, name):
                t = cpool.tile(shape, F32, tag=name)
                nc.sync.dma_start(out=t[:], in_=src)
                return t

            w1_sb = load_const([F, SE_D], w1[:], "w1_sb")
            b1_sb = load_const([SE_D, 1], b1[:], "b1_sb")
            w2_sb = load_const([SE_D, SE_D], w2[:], "w2_sb")
            b2_sb = load_const([SE_D, 1], b2[:], "b2_sb")
            wop_sb = load_const([SE_D, F], wop[:], "wop_sb")
            bop_sb = load_const([F, 1], bop[:], "bop_sb")
            wge_sb = load_const([F + 1, H], wge[:], "wge_sb")
            wf1_sb = load_const([H, FC1], wf1[:], "wf1_sb")
            bf1r_sb = load_const([1, FC1], bf1r[:], "bf1r_sb")
            wf2a_sb = load_const([H, FC2], wf2[0], "wf2a_sb")
            wf2b_sb = load_const([H, FC2], wf2[1], "wf2b_sb")
            bf2r_sb = load_const([1, FC2], bf2r[:], "bf2r_sb")
            wo_sb = load_const([FC2, OUT], wo[:], "wo_sb")
            bor_sb = load_const([1, OUT], bor[:], "bor_sb")
            g2_sb = load_const([H, 2], g2c[:], "g2_sb")
            be2_sb = load_const([H, 2], be2c[:], "be2_sb")
            g3_sb = load_const([FC2, 1], g3c[:], "g3_sb")
            be3_sb = load_const([FC2, 1], be3c[:], "be3_sb")

            # ---- phase G: SE gate (max-pool over nodes + tiny MLP) ----
            gates = []
            if "G" in phases:
                # sharded x-scan: each core reduces its N/8 slice, then
                # AllReduce(max) of the [128, BF] partials
                rows = N // NCORES
                xs_sb = cpool.tile([128, rows // 128, BF], F16)
                nc.sync.dma_start(
                    out=xs_sb[:],
                    in_=xs[:].rearrange("(p c) w -> p c w", p=128))
                redpart = cpool.tile([128, BF], F32)
                nc.vector.tensor_reduce(
                    out=redpart[:], in_=xs_sb[:].rearrange("p c w -> p w c"),
                    axis=AX.X, op=ALU.max,
                )
                # fold to per-batch pvec partials locally, tiny AllReduce
                pp = cpool.tile([F, B], F32)
                for b in range(B):
                    red_ps = psum.tile([F, 128], F32, space="PSUM", tag="ps_a")
                    nc.tensor.transpose(
                        out=red_ps[:], in_=redpart[:, b * F:(b + 1) * F],
                        identity=ident[:])
                    nc.vector.tensor_reduce(out=pp[:, b:b + 1], in_=red_ps[:],
                                            axis=AX.X, op=ALU.max)
                if skip_collective:
                    ppf = pp
                else:
                    r_in = dpool.tile([F, B], F32)
                    r_out = dpool.tile([F, B], F32)
                    nc.gpsimd.dma_start(out=r_in[:], in_=pp[:])
                    nc.gpsimd.collective_compute(
                        "AllReduce", ALU.max,
                        replica_groups=[list(range(NCORES))],
                        ins=[r_in.opt()], outs=[r_out.opt()],
                    )
                    ppf = cpool.tile([F, B], F32)
                    nc.sync.dma_start(out=ppf[:], in_=r_out[:])
                for b in range(B):
                    a1_ps = psum.tile([SE_D, 1], F32, space="PSUM", tag="ps_b")
                    nc.tensor.matmul(out=a1_ps[:], lhsT=w1_sb[:],
                                     rhs=ppf[:, b:b + 1], start=True, stop=True)
                    a1 = pool.tile([SE_D, 1], F32, tag="a1")
                    nc.scalar.activation(out=a1[:], in_=a1_ps[:], func=AF.Relu,
                                         bias=b1_sb[:])
                    a2_ps = psum.tile([SE_D, 1], F32, space="PSUM", tag="ps_b")
                    nc.tensor.matmul(out=a2_ps[:], lhsT=w2_sb[:], rhs=a1[:],
                                     start=True, stop=True)
                    a2 = pool.tile([SE_D, 1], F32, tag="a2")
                    nc.scalar.activation(out=a2[:], in_=a2_ps[:], func=AF.Relu,
                                         bias=b2_sb[:])
                    g_ps = psum.tile([F, 1], F32, space="PSUM", tag="ps_b")
                    nc.tensor.matmul(out=g_ps[:], lhsT=wop_sb[:], rhs=a2[:],
                                     start=True, stop=True)
                    gate = cpool.tile([F, 1], F32, tag=f"gate_{b}")
                    nc.scalar.activation(out=gate[:], in_=g_ps[:],
                                         func=AF.Sigmoid, bias=bop_sb[:])
                    nc.vector.tensor_scalar_add(gate[:], gate[:], 1.0)
                    gates.append(gate)

            # per-batch gate-scaled Wg: gate (x) agg @ Wg == agg @ diag(gate) Wg
            wgeb = []
            for b in range(B):
                wb = cpool.tile([F + 1, H], F32, tag=f"wgeb_{b}",
                                name=f"wgeb_{b}")
                if gates:
                    nc.vector.tensor_scalar(
                        out=wb[0:F, :], in0=wge_sb[0:F, :],
                        scalar1=gates[b][:, 0:1], scalar2=None, op0=ALU.mult)
                else:
                    nc.vector.tensor_copy(out=wb[0:F, :], in_=wge_sb[0:F, :])
                nc.vector.tensor_copy(out=wb[F:F + 1, :],
                                      in_=wge_sb[F:F + 1, :])
                wgeb.append(wb)

            # ---- phase B: SpMM + Wg + BN1 + pool partial ----
            pooled = cpool.tile([H, B], F32)
            nc.vector.memset(pooled[:], -1e30)
            poolacc = [cpool.tile([128, BF], F32, tag=f"poolacc_{g}",
                                  name=f"poolacc_{g}")
                       for g in range(2)]
            for g in range(2):
                nc.vector.memset(poolacc[g][:], -1e30)

            for t in range(TPC) if "B" in phases else []:
                cpt_t = cpts[t]
                slots_t = slots_i[t]
                off_t = int(offs_i[t])
                gidx_sb = pool.tile([128, slots_t // 16], I16, tag="gidx_sb",
                                    bufs=3)
                nc.sync.dma_start(out=gidx_sb[:],
                                  in_=gidx[:, off_t // 16:(off_t + slots_t) // 16])
                s_sb = pool.tile([128, slots_t], F16, tag="s_sb", bufs=3)
                nc.sync.dma_start(out=s_sb[:],
                                  in_=smat[:, off_t:off_t + slots_t])
                bn1_sb = pool.tile([128, 2], F32, tag="bn1_sb")
                nc.sync.dma_start(out=bn1_sb[:], in_=bn1p[t])

                msg = pool.tile([128, cpt_t, BF], F16, tag="msg", bufs=3)
                for s0 in range(0, slots_t, MAX_GATHER):
                    n_i = min(MAX_GATHER, slots_t - s0)
                    nc.gpsimd.dma_gather(
                        out_ap=msg[:, s0 // 128:(s0 + n_i) // 128, :],
                        in_ap=xt[:],
                        idxs_ap=gidx_sb[:, s0 // 16:(s0 + n_i) // 16],
                        num_idxs=n_i, num_idxs_reg=n_i, elem_size=BF,
                    )
                agg_ps = psum.tile([128, BF], F32, space="PSUM", tag="ps_agg")
                for k in range(cpt_t):
                    nc.tensor.matmul(
                        out=agg_ps[:],
                        lhsT=s_sb[:, k * 128:(k + 1) * 128],
                        rhs=msg[:, k, :],
                        start=(k == 0), stop=(k == cpt_t - 1),
                    )
                agg_sb = pool.tile([128, BF], F32, tag="agg_sb")
                nc.scalar.activation(out=agg_sb[:], in_=agg_ps[:],
                                     func=AF.Copy)
                # transpose per batch; fuse gate multiply into PSUM evacuation
                aggT = pool.tile([F + 1, B, 128], F32, tag="aggT", bufs=4)
                nc.vector.memset(aggT[F:F + 1, :, :], 1.0)
                for b in range(B):
                    tr_ps = psum.tile([F, 128], F32, space="PSUM", tag="ps_a")
                    nc.tensor.transpose(
                        out=tr_ps[:], in_=agg_sb[:, b * F:(b + 1) * F],
                        identity=ident[:])
                    nc.vector.tensor_copy(out=aggT[0:F, b, :], in_=tr_ps[:])
                # h2 = relu(agg_gated @ Wg + bg), grouped 4 batches per PSUM bank
                sums = pool.tile([128, 2], F32, tag="sums")
                sqs = pool.tile([128, 2], F32, tag="sqs")
                h2g = []
                for g in range(2):
                    h2_ps = psum.tile([128, BF], F32, space="PSUM", tag="ps_h2")
                    for j in range(4):
                        b = g * 4 + j
                        nc.tensor.matmul(
                            out=h2_ps[:, j * H:(j + 1) * H],
                            lhsT=aggT[:, b, :], rhs=wgeb[b][:],
                            start=True, stop=True)
                    h2 = pool.tile([128, BF], F32, tag=f"h2_{g}")
                    nc.scalar.activation(out=h2[:], in_=h2_ps[:], func=AF.Relu,
                                         accum_out=sums[:, g:g + 1])
                    sqscr = pool.tile([128, BF], F32, tag="sqscr")
                    nc.scalar.activation(out=sqscr[:], in_=h2[:],
                                         func=AF.Square,
                                         accum_out=sqs[:, g:g + 1])
                    h2g.append(h2)
                # BN1 per-node affine
                rsumt = pool.tile([128, 1], F32, tag="rsumt")
                nc.vector.tensor_reduce(out=rsumt[:], in_=sums[:], axis=AX.X,
                                        op=ALU.add)
                sqsumt = pool.tile([128, 1], F32, tag="sqsumt")
                nc.vector.tensor_reduce(out=sqsumt[:], in_=sqs[:], axis=AX.X,
                                        op=ALU.add)
                mean = pool.tile([128, 1], F32, tag="mean")
                nc.vector.tensor_scalar_mul(mean[:], rsumt[:], 1.0 / (B * H))
                msq = pool.tile([128, 1], F32, tag="msq")
                nc.vector.tensor_scalar_mul(msq[:], sqsumt[:], 1.0 / (B * H))
                var = pool.tile([128, 1], F32, tag="var")
                nc.vector.tensor_tensor(out=var[:], in0=mean[:], in1=mean[:],
                                        op=ALU.mult)
                nc.vector.tensor_tensor(out=var[:], in0=msq[:], in1=var[:],
                                        op=ALU.subtract)
                nc.vector.tensor_scalar_add(var[:], var[:], BN_EPS)
                inv = pool.tile([128, 1], F32, tag="inv")
                nc.vector.reciprocal(out=inv[:], in_=var[:])
                rstd = pool.tile([128, 1], F32, tag="rstd")
                nc.scalar.sqrt(out=rstd[:], in_=inv[:])
                aco = pool.tile([128, 1], F32, tag="aco")
                nc.vector.tensor_tensor(out=aco[:], in0=rstd[:],
                                        in1=bn1_sb[:, 0:1], op=ALU.mult)
                bco = pool.tile([128, 1], F32, tag="bco")
                nc.vector.tensor_tensor(out=bco[:], in0=mean[:], in1=aco[:],
                                        op=ALU.mult)
                nc.vector.tensor_tensor(out=bco[:], in0=bn1_sb[:, 1:2],
                                        in1=bco[:], op=ALU.subtract)
                for g in range(2):
                    h2n = pool.tile([128, BF], F32, tag="h2n")
                    nc.vector.tensor_scalar(
                        out=h2n[:], in0=h2g[g][:],
                        scalar1=aco[:, 0:1], scalar2=bco[:, 0:1],
                        op0=ALU.mult, op1=ALU.add)
                    nc.vector.tensor_tensor(out=poolacc[g][:],
                                            in0=poolacc[g][:], in1=h2n[:],
                                            op=ALU.max)

            # fold pooled partials: per batch, transpose + reduce over nodes
            for b in range(B):
                g, j = b // 4, b % 4
                hT_ps = psum.tile([128, 128], F32, space="PSUM", tag="ps_a")
                nc.tensor.transpose(
                    out=hT_ps[:], in_=poolacc[g][:, j * H:(j + 1) * H],
                    identity=ident[:])
                nc.vector.tensor_reduce(out=pooled[:, b:b + 1], in_=hT_ps[:],
                                        axis=AX.X, op=ALU.max)

            # ---- phase C: AllReduce(max) + replicated head ----
            if skip_collective:
                pooledf = pooled
            else:
                p_in = dpool.tile([H, B], F32)
                p_out = dpool.tile([H, B], F32)
                nc.gpsimd.dma_start(out=p_in[:], in_=pooled[:])
                nc.gpsimd.collective_compute(
                    "AllReduce", ALU.max,
                    replica_groups=[list(range(NCORES))],
                    ins=[p_in.opt()], outs=[p_out.opt()],
                )
                pooledf = cpool.tile([H, B], F32)
                nc.sync.dma_start(out=pooledf[:], in_=p_out[:])

            def bn_free8(z, nrows, gamma_col, beta_col, tag):
                """BN over the 8 free-dim entries of z [nrows, 8] -> new tile."""
                rs = pool.tile([nrows, 1], F32, tag=f"{tag}_rs")
                nc.vector.tensor_reduce(out=rs[:], in_=z[:], axis=AX.X,
                                        op=ALU.add)
                nc.vector.tensor_scalar_mul(rs[:], rs[:], 1.0 / B)
                sqt = pool.tile([nrows, B], F32, tag=f"{tag}_sqt")
                sq = pool.tile([nrows, 1], F32, tag=f"{tag}_sq")
                nc.scalar.activation(out=sqt[:], in_=z[:], func=AF.Square,
                                     accum_out=sq[:])
                nc.vector.tensor_scalar_mul(sq[:], sq[:], 1.0 / B)
                v = pool.tile([nrows, 1], F32, tag=f"{tag}_v")
                nc.vector.tensor_tensor(out=v[:], in0=rs[:], in1=rs[:],
                                        op=ALU.mult)
                nc.vector.tensor_tensor(out=v[:], in0=sq[:], in1=v[:],
                                        op=ALU.subtract)
                nc.vector.tensor_scalar_add(v[:], v[:], BN_EPS)
                iv = pool.tile([nrows, 1], F32, tag=f"{tag}_iv")
                nc.vector.reciprocal(out=iv[:], in_=v[:])
                rst = pool.tile([nrows, 1], F32, tag=f"{tag}_rst")
                nc.scalar.sqrt(out=rst[:], in_=iv[:])
                ac = pool.tile([nrows, 1], F32, tag=f"{tag}_ac")
                nc.vector.tensor_tensor(out=ac[:], in0=rst[:], in1=gamma_col,
                                        op=ALU.mult)
                bc = pool.tile([nrows, 1], F32, tag=f"{tag}_bc")
                nc.vector.tensor_tensor(out=bc[:], in0=rs[:], in1=ac[:],
                                        op=ALU.mult)
                nc.vector.tensor_tensor(out=bc[:], in0=beta_col, in1=bc[:],
                                        op=ALU.subtract)
                zn = pool.tile([nrows, B], F32, tag=f"{tag}_zn")
                nc.vector.tensor_scalar(out=zn[:], in0=z[:],
                                        scalar1=ac[:, 0:1], scalar2=bc[:, 0:1],
                                        op0=ALU.mult, op1=ALU.add)
                return zn

            # fc1
            z1_ps = psum.tile([B, FC1], F32, space="PSUM", tag="ps_b")
            nc.tensor.matmul(out=z1_ps[:], lhsT=ones_r[0:1, 0:B],
                             rhs=bf1r_sb[:], start=True, stop=False)
            nc.tensor.matmul(out=z1_ps[:], lhsT=pooledf[:], rhs=wf1_sb[:],
                             start=False, stop=True)
            z1 = pool.tile([B, FC1], F32, tag="z1")
            nc.scalar.activation(out=z1[:], in_=z1_ps[:], func=AF.Relu)
            z1n = []
            for j in range(2):
                zT_ps = psum.tile([128, B], F32, space="PSUM", tag="ps_a")
                nc.tensor.transpose(out=zT_ps[:],
                                    in_=z1[:, j * 128:(j + 1) * 128],
                                    identity=ident[0:B, 0:B])
                zT = pool.tile([128, B], F32, tag=f"z1T_{j}")
                nc.vector.tensor_copy(out=zT[:], in_=zT_ps[:])
                z1n.append(bn_free8(zT, 128, g2_sb[:, j:j + 1],
                                    be2_sb[:, j:j + 1], f"bn2_{j}"))
            # fc2
            z2_ps = psum.tile([B, FC2], F32, space="PSUM", tag="ps_b")
            nc.tensor.matmul(out=z2_ps[:], lhsT=ones_r[0:1, 0:B],
                             rhs=bf2r_sb[:], start=True, stop=False)
            nc.tensor.matmul(out=z2_ps[:], lhsT=z1n[0][:], rhs=wf2a_sb[:],
                             start=False, stop=False)
            nc.tensor.matmul(out=z2_ps[:], lhsT=z1n[1][:], rhs=wf2b_sb[:],
                             start=False, stop=True)
            z2 = pool.tile([B, FC2], F32, tag="z2")
            nc.scalar.activation(out=z2[:], in_=z2_ps[:], func=AF.Relu)
            z2T_ps = psum.tile([FC2, B], F32, space="PSUM", tag="ps_a")
            nc.tensor.transpose(out=z2T_ps[:], in_=z2[:],
                                identity=ident[0:B, 0:B])
            z2T = pool.tile([FC2, B], F32, tag="z2T")
            nc.vector.tensor_copy(out=z2T[:], in_=z2T_ps[:])
            z2n = bn_free8(z2T, FC2, g3_sb[:, 0:1], be3_sb[:, 0:1], "bn3")
            # logits + softmax
            lg_ps = psum.tile([B, OUT], F32, space="PSUM", tag="ps_b")
            nc.tensor.matmul(out=lg_ps[:], lhsT=ones_r[0:1, 0:B],
                             rhs=bor_sb[:], start=True, stop=False)
            nc.tensor.matmul(out=lg_ps[:], lhsT=z2n[:], rhs=wo_sb[:],
                             start=False, stop=True)
            lg = pool.tile([B, OUT], F32, tag="lg")
            nc.vector.tensor_copy(out=lg[:], in_=lg_ps[:])
            mx = pool.tile([B, 1], F32, tag="mx")
            nc.vector.tensor_reduce(out=mx[:], in_=lg[:], axis=AX.X, op=ALU.max)
            ex = pool.tile([B, OUT], F32, tag="ex")
            nc.vector.tensor_scalar(out=ex[:], in0=lg[:], scalar1=mx[:, 0:1],
                                    scalar2=None, op0=ALU.subtract)
            nc.scalar.activation(out=ex[:], in_=ex[:], func=AF.Exp)
            ssum = pool.tile([B, 1], F32, tag="ssum")
            nc.vector.tensor_reduce(out=ssum[:], in_=ex[:], axis=AX.X,
                                    op=ALU.add)
            sinv = pool.tile([B, 1], F32, tag="sinv")
            nc.vector.reciprocal(out=sinv[:], in_=ssum[:])
            sm = pool.tile([B, OUT], F32, tag="sm")
            nc.vector.tensor_scalar(out=sm[:], in0=ex[:], scalar1=sinv[:, 0:1],
                                    scalar2=None, op0=ALU.mult)
            nc.sync.dma_start(out=out_t[:], in_=sm[:])
    nc.compile()
    return nc


def preprocess(x, src, dst, edge_w):
    """Host marshalling: node-major x table + sorted/padded edge tiles."""
    order = np.argsort(dst, kind="stable")
    ss = src[order].astype(np.int64)
    ds = dst[order].astype(np.int64)
    ws = edge_w[order].astype(np.float32)
    tile_id = ds // 128
    dloc = ds % 128
    counts = np.bincount(tile_id, minlength=NTILE)
    cpt = int(np.ceil(counts.max() / 128))
    slots = cpt * 128

    gidx_all = np.zeros((NTILE, slots), np.int16)
    sval = np.zeros((NTILE, slots), np.float32)
    sloc = np.zeros((NTILE, slots), np.int64)
    offs = np.concatenate([[0], np.cumsum(counts)])
    for t in range(NTILE):
        cnt = counts[t]
        seg = slice(offs[t], offs[t + 1])
        # order each tile's edges by ascending src so gather descriptors
        # read ascending HBM addresses (DRAM row-buffer locality); the
        # segment-sum is order-invariant since S follows the slot order
        o = np.argsort(ss[seg], kind="stable")
        gidx_all[t, :cnt] = ss[seg][o]
        sval[t, :cnt] = ws[seg][o]
        sloc[t, :cnt] = dloc[seg][o]

    # one-hot blocks: smat[t][e, k*128 + d] = w for edge slot i = k*128+e
    smat = np.zeros((NTILE, 128, slots), np.float32)
    i = np.arange(slots)
    kk, ee = i // 128, i % 128
    for t in range(NTILE):
        smat[t, ee, kk * 128 + sloc[t]] = sval[t]
    smat = smat.astype(f16)

    # wrapped int16 index tables: [16, slots//16] replicated to 128 partitions
    gidx_w = np.zeros((NTILE, 128, slots // 16), np.int16)
    for t in range(NTILE):
        base = gidx_all[t].reshape(slots // 16, 16).T
        gidx_w[t] = np.tile(base, (8, 1))

    # per-core tile order (descending edge count) and per-slot chunk counts
    order_pc = np.zeros((NCORES, TPC), np.int64)
    for c in range(NCORES):
        tl = np.arange(c * TPC, (c + 1) * TPC)
        order_pc[c] = tl[np.argsort(-counts[tl], kind="stable")]
    cpts = tuple(
        int(np.ceil(max(counts[order_pc[c][i]] for c in range(NCORES)) / 128))
        for i in range(TPC)
    )
    xt = np.ascontiguousarray(
        np.asarray(x, np.float32).transpose(1, 0, 2).reshape(N, BF)
    ).astype(f16)
    return xt, gidx_w, smat, cpts, order_pc


def make_in_maps(inputs, xt, gidx_w, smat, cpts, order_pc):
    g1 = np.asarray(inputs["g1"], np.float32).reshape(NTILE, 128)
    beta1 = np.asarray(inputs["beta1"], np.float32).reshape(NTILE, 128)
    bn1 = np.stack([g1, beta1], axis=-1)  # [NTILE, 128, 2]

    f32 = lambda a: np.ascontiguousarray(np.asarray(a, np.float32))
    wge = np.concatenate(
        [f32(inputs["Wg"]), f32(inputs["bg"]).reshape(1, H)], axis=0)
    shared = {
        "xt": xt,
        "w1": f32(inputs["W1"]),
        "b1": f32(inputs["b1"]).reshape(SE_D, 1),
        "w2": f32(inputs["W2"]),
        "b2": f32(inputs["b2"]).reshape(SE_D, 1),
        "wop": f32(inputs["Wop"]),
        "bop": f32(inputs["bop"]).reshape(F, 1),
        "wge": wge,
        "wf1": f32(inputs["Wf1"]),
        "bf1r": f32(inputs["bf1"]).reshape(1, FC1),
        "wf2": f32(inputs["Wf2"]).reshape(2, H, FC2),
        "bf2r": f32(inputs["bf2"]).reshape(1, FC2),
        "wo": f32(inputs["Wo"]),
        "bor": f32(inputs["bo"]).reshape(1, OUT),
        "g2c": f32(inputs["g2"]).reshape(2, H).T.copy(),
        "be2c": f32(inputs["beta2"]).reshape(2, H).T.copy(),
        "g3c": f32(inputs["g3"]).reshape(FC2, 1),
        "be3c": f32(inputs["beta3"]).reshape(FC2, 1),
    }
    in_maps = []
    for c in range(NCORES):
        order = order_pc[c]
        m = dict(shared)
        m["xs"] = np.ascontiguousarray(xt[c * (N // NCORES):(c + 1) * (N // NCORES)])
        m["gidx"] = np.ascontiguousarray(np.concatenate(
            [gidx_w[gt][:, :cpts[i] * 8] for i, gt in enumerate(order)], axis=1))
        m["smat"] = np.ascontiguousarray(np.concatenate(
            [smat[gt][:, :cpts[i] * 128] for i, gt in enumerate(order)], axis=1))
        m["bn1p"] = np.ascontiguousarray(bn1[order])
        in_maps.append(m)
    return in_maps


_CACHE = {}
LAST_RESULT = None  # BassKernelResults of the most recent kernel() call


def kernel(**inputs):
    global LAST_RESULT
    xt, gidx_w, smat, cpts, order_pc = preprocess(
        np.asarray(inputs["x"]), np.asarray(inputs["src"]),
        np.asarray(inputs["dst"]), np.asarray(inputs["edge_w"]))
    if cpts not in _CACHE:
        _CACHE[cpts] = build_kernel(cpts)
    nc = _CACHE[cpts]
    in_maps = make_in_maps(inputs, xt, gidx_w, smat, cpts, order_pc)
    trace = os.environ.get("BASS_KERNEL_TRACE", "0") == "1"
    res = run_bass_kernel_spmd(nc, in_maps, list(range(NCORES)), trace=trace)
    LAST_RESULT = res
    return np.asarray(res.results[0]["out"], np.float32)

